# revision 9
# baseline (speedup 1.0000x reference)
"""Trainium2 Bass kernel for nn_CAREModel (graph CVAE encoder + losses + topk relations).

Sharding: node dim N=4096 row-sharded over 8 cores (512 rows each).
Device per core: s = ns@Wh (shard, fp32) -> AllGather -> gcn1 (fp32r, adjT via
PE transpose) -> MHA (fp32) -> t = h2@[Wmu|Wvar] -> AllGather -> gcn2 (fp32r)
-> z -> AllGather zT -> z@zT (fp32r) feeding BCE partial sums (ACT ls-pipeline
+ DVE stt accums + PE V=z^T@labels / colsum matmuls) + KLD partials.
Host: merges partials into losses; top-512 selection from z (sigmoid of the
logits saturates to 1.0f, so jax.lax.top_k returns the first 512 flat indices
with logit above the fp32 saturation threshold); relations = m[i]+m[j].
"""
import os, sys
for _p in ("/opt/trn_rl_repo", "/root/.axon_site/_ro/trn_rl_repo"):
    if os.path.isdir(_p) and _p not in sys.path:
        sys.path.insert(0, _p)

import numpy as np
import ml_dtypes

N = 4096
S = 512          # shard rows per core
NC = 8           # cores
F_IN = 300
H1 = 256
H2 = 128
C_LEN = 64
DK = 64
HEADS = 2
DKH = DK // HEADS  # 32
MAX_K = 512
NKC = N // 128   # 32 contraction chunks
NRT = S // 128   # 4 row tiles per shard
NCC = N // 512   # 8 column chunks

_COMPILED = None
_last_in_maps = None


def _build():
    from concourse import bacc, mybir, tile
    from concourse.masks import make_identity

    F32, F32R, BF16 = mybir.dt.float32, mybir.dt.float32r, mybir.dt.bfloat16
    AF = mybir.ActivationFunctionType
    ALU = mybir.AluOpType

    nc = bacc.Bacc("TRN2", target_bir_lowering=False, debug=False, num_devices=NC)

    def din(name, shape, dt=F32):
        return nc.dram_tensor(name, shape, dt, kind="ExternalInput").ap()

    def dout(name, shape, dt=F32):
        return nc.dram_tensor(name, shape, dt, kind="ExternalOutput").ap()

    d_adj = din("adj_s", [S, N])
    d_adjp = din("adjp_s", [S, N])
    d_lab = din("lab_s", [S, N], BF16)
    d_ns = din("ns_s", [S, F_IN])
    d_eps = din("eps_s", [S, H2])
    d_cond = din("cond", [C_LEN, H1])
    d_W = {}
    for e in ("p", "r"):
        d_W[f"Wh_{e}"] = din(f"Wh_{e}", [F_IN, H1])
        d_W[f"Wq_{e}"] = din(f"Wq_{e}", [H1, DK])
        d_W[f"Wk_{e}"] = din(f"Wk_{e}", [H1, DK])
        d_W[f"Wv_{e}"] = din(f"Wv_{e}", [H1, DK])
        d_W[f"Wo_{e}"] = din(f"Wo_{e}", [DK, H1])
        d_W[f"Wmv_{e}"] = din(f"Wmv_{e}", [H1, 2 * H2])
    d_Wmap = din("Wmap", [F_IN, H2])

    o_z = dout("z_out", [S, H2])
    o_m = dout("m_out", [S, H2])
    o_sx = dout("sx", [128, 32])
    o_sg = dout("sg", [128, 32])
    o_slx = dout("slx", [128, 32])
    o_slg = dout("slg", [128, 32])
    o_kld = dout("kldp", [128, NRT])
    o_labc = dout("labcol", [1, N])

    ag_in = {}
    ag_out = {}
    for nm, shp in (("sp", [S, H1]), ("sr", [S, H1]),
                    ("tp", [S, 2 * H2]), ("tr", [S, 2 * H2])):
        ag_in[nm] = nc.dram_tensor(f"agi_{nm}", shp, F32).ap()
        ag_out[nm] = nc.dram_tensor(f"ago_{nm}", [shp[0] * NC, shp[1]], F32,
                                    addr_space="Shared").ap()
    ag_in["z"] = nc.dram_tensor("agi_z", [H2, S], F32).ap()
    ag_out["z"] = nc.dram_tensor("ago_z", [H2 * NC, S], F32,
                                 addr_space="Shared").ap()

    RG = [list(range(NC))]

    with tile.TileContext(nc) as tc:
        with tc.tile_pool(name="const", bufs=1) as cpool, \
             tc.tile_pool(name="big", bufs=1) as bigpool, \
             tc.tile_pool(name="stg", bufs=2) as stg, \
             tc.tile_pool(name="lab", bufs=3) as labp, \
             tc.tile_pool(name="wk1", bufs=1) as wk1, \
             tc.tile_pool(name="wk2", bufs=2) as wk2, \
             tc.tile_pool(name="accs", bufs=1) as accp, \
             tc.tile_pool(name="pt", bufs=2, space="PSUM") as pt, \
             tc.tile_pool(name="pg", bufs=3, space="PSUM") as pg, \
             tc.tile_pool(name="pv", bufs=2, space="PSUM") as pv, \
             tc.tile_pool(name="pl", bufs=1, space="PSUM") as pl:

            ident = cpool.tile([128, 128], F32, tag="ident")
            make_identity(nc, ident)

            # ---------------- weights + small inputs ----------------
            Wsb = {}
            for e in ("p", "r"):
                w = cpool.tile([128, 3 * H1], F32, tag=f"Wh_{e}")
                for kc in range(3):
                    rows = min(128, F_IN - kc * 128)
                    nc.sync.dma_start(out=w[:rows, kc * H1:kc * H1 + H1],
                                      in_=d_W[f"Wh_{e}"][kc * 128:kc * 128 + rows, :])
                Wsb[f"Wh_{e}"] = w
                for wn in ("Wq", "Wk", "Wv"):
                    w = cpool.tile([128, 2 * DK], F32, tag=f"{wn}_{e}")
                    for kc in range(2):
                        nc.sync.dma_start(out=w[:, kc * DK:kc * DK + DK],
                                          in_=d_W[f"{wn}_{e}"][kc * 128:(kc + 1) * 128, :])
                    Wsb[f"{wn}_{e}"] = w
                w = cpool.tile([DK, H1], F32, tag=f"Wo_{e}")
                nc.sync.dma_start(out=w[:], in_=d_W[f"Wo_{e}"][:])
                Wsb[f"Wo_{e}"] = w
                w = cpool.tile([128, 2 * 2 * H2], F32, tag=f"Wmv_{e}")
                for kc in range(2):
                    nc.sync.dma_start(out=w[:, kc * 2 * H2:(kc + 1) * 2 * H2],
                                      in_=d_W[f"Wmv_{e}"][kc * 128:(kc + 1) * 128, :])
                Wsb[f"Wmv_{e}"] = w
            wmap = cpool.tile([128, 3 * H2], F32, tag="Wmap")
            for kc in range(3):
                rows = min(128, F_IN - kc * 128)
                nc.sync.dma_start(out=wmap[:rows, kc * H2:kc * H2 + H2],
                                  in_=d_Wmap[kc * 128:kc * 128 + rows, :])
            cond_sb = cpool.tile([C_LEN, H1], F32, tag="cond")
            nc.sync.dma_start(out=cond_sb[:], in_=d_cond[:])
            eps_sb = cpool.tile([128, NRT * H2], F32, tag="eps")
            for rt in range(NRT):
                nc.sync.dma_start(out=eps_sb[:, rt * H2:(rt + 1) * H2],
                                  in_=d_eps[rt * 128:(rt + 1) * 128, :])

            # condT [H1(2 chunks of 128 rows), C_LEN]
            condT = cpool.tile([128, 2 * C_LEN], F32, tag="condT")
            for kc in range(2):
                ps = pt.tile([128, 128], F32, tag="pt")
                nc.tensor.transpose(ps[:, :C_LEN],
                                    cond_sb[:, kc * 128:(kc + 1) * 128],
                                    ident[:C_LEN, :C_LEN])
                nc.scalar.copy(condT[:, kc * C_LEN:(kc + 1) * C_LEN],
                               ps[:, :C_LEN])

            # ---------------- nsT + s + m ----------------
            nsT = cpool.tile([128, 3 * S], F32, tag="nsT")
            for rt in range(NRT):
                nrow = stg.tile([128, F_IN], F32, tag="xstg")
                nc.sync.dma_start(out=nrow[:], in_=d_ns[rt * 128:(rt + 1) * 128, :])
                for kc in range(3):
                    rows = min(128, F_IN - kc * 128)
                    ps = pt.tile([128, 128], F32, tag="pt")
                    nc.tensor.transpose(ps[:rows, :],
                                        nrow[:, kc * 128:kc * 128 + rows],
                                        ident[:])
                    nc.scalar.copy(nsT[:rows, kc * S + rt * 128:kc * S + (rt + 1) * 128],
                                   ps[:rows, :128])

            for e in ("p", "r"):
                for rt in range(NRT):
                    ps = pg.tile([128, 512], F32, tag="pg")
                    for kc in range(3):
                        rows = min(128, F_IN - kc * 128)
                        nc.tensor.matmul(
                            ps[:, :H1],
                            nsT[:rows, kc * S + rt * 128:kc * S + (rt + 1) * 128],
                            Wsb[f"Wh_{e}"][:rows, kc * H1:(kc + 1) * H1],
                            start=(kc == 0), stop=(kc == 2))
                    so = wk1.tile([128, H1], F32, tag="s_out")
                    nc.scalar.activation(so[:], ps[:, :H1], AF.Prelu, alpha=0.01)
                    nc.sync.dma_start(out=ag_in["s" + e][rt * 128:(rt + 1) * 128, :],
                                      in_=so[:])
            nc.gpsimd.collective_compute("AllGather", ALU.bypass, ins=[ag_in["sp"][:]],
                                         outs=[ag_out["sp"][:]], replica_groups=RG)
            nc.gpsimd.collective_compute("AllGather", ALU.bypass, ins=[ag_in["sr"][:]],
                                         outs=[ag_out["sr"][:]], replica_groups=RG)

            for rt in range(NRT):
                ps = pg.tile([128, 512], F32, tag="pg")
                for kc in range(3):
                    rows = min(128, F_IN - kc * 128)
                    nc.tensor.matmul(
                        ps[:, :H2],
                        nsT[:rows, kc * S + rt * 128:kc * S + (rt + 1) * 128],
                        wmap[:rows, kc * H2:(kc + 1) * H2],
                        start=(kc == 0), stop=(kc == 2))
                mo = wk1.tile([128, H2], F32, tag="m_out")
                nc.scalar.activation(mo[:], ps[:, :H2], AF.Prelu, alpha=0.01)
                nc.sync.dma_start(out=o_m[rt * 128:(rt + 1) * 128, :], in_=mo[:])

            # ---------------- shared state ----------------
            zsb = cpool.tile([128, NRT * H2], F32, tag="zsb")
            zsT_r = cpool.tile([128, S], F32R, tag="zsT")
            muv_p_t = cpool.tile([128, NRT * 2 * H2], F32, tag="muv_p")
            muv_r_t = cpool.tile([128, NRT * 2 * H2], F32, tag="muv_r")
            muv = {"p": muv_p_t, "r": muv_r_t}

            def encoder(e, d_adj_e):
                adjT = bigpool.tile([128, NKC * S], F32R, tag="adjT")
                for rt in range(NRT):
                    for blk in range(4):
                        st = stg.tile([128, 1024], F32, tag="adjstg")
                        nc.sync.dma_start(
                            out=st[:],
                            in_=d_adj_e[rt * 128:(rt + 1) * 128,
                                        blk * 1024:(blk + 1) * 1024])
                        for q in range(8):
                            kc = blk * 8 + q
                            ps = pt.tile([128, 128], F32, tag="pt")
                            nc.tensor.transpose(ps[:], st[:, q * 128:(q + 1) * 128],
                                                ident[:])
                            nc.scalar.copy(
                                adjT[:, kc * S + rt * 128:kc * S + (rt + 1) * 128],
                                ps[:])

                X = bigpool.tile([128, NKC * H1], F32R, tag="bigX")
                agv = ag_out["s" + e].rearrange("(kc p) n -> kc p n", p=128)
                for kc in range(NKC):
                    st = stg.tile([128, H1], F32, tag="xstg")
                    nc.sync.dma_start(out=st[:], in_=agv[kc])
                    nc.vector.tensor_copy(X[:, kc * H1:(kc + 1) * H1], st[:])

                h1T = wk1.tile([128, 2 * S], F32, tag="h1T")
                for mh in range(2):
                    ps = pg.tile([128, 512], F32, tag="pg")
                    for kc in range(NKC):
                        nc.tensor.matmul(
                            ps[:],
                            X[:, kc * H1 + mh * 128:kc * H1 + (mh + 1) * 128],
                            adjT[:, kc * S:(kc + 1) * S],
                            start=(kc == 0), stop=(kc == NKC - 1))
                    nc.scalar.activation(h1T[:, mh * S:(mh + 1) * S], ps[:],
                                         AF.Prelu, alpha=0.01)

                # MHA
                kT = wk1.tile([DK, C_LEN], F32, tag="kT")
                ps = pt.tile([128, 128], F32, tag="pt")
                for kc in range(2):
                    nc.tensor.matmul(ps[:DK, :C_LEN],
                                     Wsb[f"Wk_{e}"][:, kc * DK:(kc + 1) * DK],
                                     condT[:, kc * C_LEN:(kc + 1) * C_LEN],
                                     start=(kc == 0), stop=(kc == 1))
                nc.scalar.copy(kT[:], ps[:DK, :C_LEN])
                vsb = wk1.tile([C_LEN, DK], F32, tag="vsb")
                ps = pt.tile([128, 128], F32, tag="pt")
                for kc in range(2):
                    nc.tensor.matmul(ps[:C_LEN, :DK],
                                     condT[:, kc * C_LEN:(kc + 1) * C_LEN],
                                     Wsb[f"Wv_{e}"][:, kc * DK:(kc + 1) * DK],
                                     start=(kc == 0), stop=(kc == 1))
                nc.scalar.copy(vsb[:], ps[:C_LEN, :DK])
                qT = wk1.tile([DK, S], F32, tag="qT")
                ps = pg.tile([128, 512], F32, tag="pg")
                for kc in range(2):
                    nc.tensor.matmul(ps[:DK, :], Wsb[f"Wq_{e}"][:, kc * DK:(kc + 1) * DK],
                                     h1T[:, kc * S:(kc + 1) * S],
                                     start=(kc == 0), stop=(kc == 1))
                nc.scalar.copy(qT[:], ps[:DK, :])

                attnT = wk1.tile([C_LEN, 2 * S], F32, tag="attnT")
                inv_sqrt_dk = float(1.0 / np.sqrt(np.float32(DKH)))
                for h in range(HEADS):
                    for rt in range(NRT):
                        ps = pt.tile([128, 128], F32, tag="pt")
                        nc.tensor.matmul(
                            ps[:, :C_LEN],
                            qT[h * DKH:(h + 1) * DKH, rt * 128:(rt + 1) * 128],
                            kT[h * DKH:(h + 1) * DKH, :],
                            start=True, stop=True)
                        esb = wk2.tile([128, C_LEN], F32, tag="esb")
                        ssum = wk2.tile([128, 1], F32, tag="ssum")
                        nc.scalar.activation(esb[:], ps[:, :C_LEN], AF.Exp,
                                             scale=inv_sqrt_dk, accum_out=ssum[:])
                        rcp = wk2.tile([128, 1], F32, tag="rcp")
                        nc.vector.reciprocal(rcp[:], ssum[:])
                        nc.vector.tensor_scalar_mul(esb[:], esb[:], rcp[:])
                        pst = pt.tile([128, 128], F32, tag="pt")
                        nc.tensor.transpose(pst[:C_LEN, :], esb[:], ident[:])
                        nc.scalar.copy(
                            attnT[:, h * S + rt * 128:h * S + (rt + 1) * 128],
                            pst[:C_LEN, :128])
                oT = wk1.tile([DK, S], F32, tag="oT")
                for h in range(HEADS):
                    ps = pg.tile([128, 512], F32, tag="pg")
                    nc.tensor.matmul(ps[:DKH, :], vsb[:, h * DKH:(h + 1) * DKH],
                                     attnT[:, h * S:(h + 1) * S],
                                     start=True, stop=True)
                    nc.scalar.copy(oT[h * DKH:(h + 1) * DKH, :], ps[:DKH, :])
                h2T = wk1.tile([128, 2 * S], F32, tag="h2T")
                for mh in range(2):
                    ps = pg.tile([128, 512], F32, tag="pg")
                    nc.tensor.matmul(ps[:], Wsb[f"Wo_{e}"][:, mh * 128:(mh + 1) * 128],
                                     oT[:], start=True, stop=True)
                    nc.scalar.copy(h2T[:, mh * S:(mh + 1) * S], ps[:])

                for rt in range(NRT):
                    ps = pg.tile([128, 512], F32, tag="pg")
                    for kc in range(2):
                        nc.tensor.matmul(
                            ps[:, :2 * H2],
                            h2T[:, kc * S + rt * 128:kc * S + (rt + 1) * 128],
                            Wsb[f"Wmv_{e}"][:, kc * 2 * H2:(kc + 1) * 2 * H2],
                            start=(kc == 0), stop=(kc == 1))
                    to = wk1.tile([128, 2 * H2], F32, tag="t_out")
                    nc.scalar.activation(to[:], ps[:, :2 * H2], AF.Prelu, alpha=0.01)
                    nc.sync.dma_start(out=ag_in["t" + e][rt * 128:(rt + 1) * 128, :],
                                      in_=to[:])
                nc.gpsimd.collective_compute(
                    "AllGather", ALU.bypass, ins=[ag_in["t" + e][:]],
                    outs=[ag_out["t" + e][:]], replica_groups=RG)

                T = bigpool.tile([128, NKC * 2 * H2], F32R, tag="bigX")
                agv2 = ag_out["t" + e].rearrange("(kc p) n -> kc p n", p=128)
                for kc in range(NKC):
                    st = stg.tile([128, 2 * H2], F32, tag="xstg")
                    nc.sync.dma_start(out=st[:], in_=agv2[kc])
                    nc.vector.tensor_copy(T[:, kc * 2 * H2:(kc + 1) * 2 * H2], st[:])
                for rt in range(NRT):
                    ps = pg.tile([128, 512], F32, tag="pg")
                    for kc in range(NKC):
                        nc.tensor.matmul(
                            ps[:, :2 * H2],
                            adjT[:, kc * S + rt * 128:kc * S + (rt + 1) * 128],
                            T[:, kc * 2 * H2:(kc + 1) * 2 * H2],
                            start=(kc == 0), stop=(kc == NKC - 1))
                    nc.scalar.activation(muv[e][:, rt * 2 * H2:(rt + 1) * 2 * H2],
                                         ps[:, :2 * H2], AF.Prelu, alpha=0.01)

            encoder("p", d_adj)

            # ---------------- z (posterior only) ----------------
            for rt in range(NRT):
                mu_ap = muv["p"][:, rt * 2 * H2:rt * 2 * H2 + H2]
                lv_ap = muv["p"][:, rt * 2 * H2 + H2:(rt + 1) * 2 * H2]
                u = wk1.tile([128, H2], F32, tag="z_u")
                ez = wk1.tile([128, H2], F32, tag="z_ez")
                nc.vector.tensor_scalar_mul(u[:], lv_ap, 0.5)
                nc.vector.tensor_scalar(out=ez[:], in0=u[:], scalar1=0.25,
                                        scalar2=1.0, op0=ALU.mult, op1=ALU.add)
                nc.vector.tensor_tensor(out=ez[:], in0=ez[:], in1=u[:], op=ALU.mult)
                nc.vector.tensor_scalar(out=ez[:], in0=ez[:], scalar1=1.0 / 3.0,
                                        scalar2=1.0, op0=ALU.mult, op1=ALU.add)
                nc.vector.tensor_tensor(out=ez[:], in0=ez[:], in1=u[:], op=ALU.mult)
                nc.vector.tensor_scalar(out=ez[:], in0=ez[:], scalar1=0.5,
                                        scalar2=1.0, op0=ALU.mult, op1=ALU.add)
                nc.vector.tensor_tensor(out=ez[:], in0=ez[:], in1=u[:], op=ALU.mult)
                nc.vector.tensor_scalar_add(ez[:], ez[:], 1.0)
                zt = zsb[:, rt * H2:(rt + 1) * H2]
                nc.vector.tensor_tensor(out=ez[:], in0=ez[:],
                                        in1=eps_sb[:, rt * H2:(rt + 1) * H2],
                                        op=ALU.mult)
                nc.vector.tensor_tensor(out=zt, in0=ez[:], in1=mu_ap, op=ALU.add)
                nc.sync.dma_start(out=o_z[rt * 128:(rt + 1) * 128, :], in_=zt)
                ps = pt.tile([128, 128], F32, tag="pt")
                nc.tensor.transpose(ps[:H2, :], zt, ident[:])
                zT32 = wk1.tile([H2, 128], F32, tag="zT32")
                nc.scalar.copy(zT32[:], ps[:H2, :128])
                nc.sync.dma_start(out=ag_in["z"][:, rt * 128:(rt + 1) * 128],
                                  in_=zT32[:])
                nc.vector.tensor_copy(zsT_r[:H2, rt * 128:(rt + 1) * 128],
                                      ps[:H2, :128])
            nc.gpsimd.collective_compute("AllGather", ALU.bypass, ins=[ag_in["z"][:]],
                                         outs=[ag_out["z"][:]], replica_groups=RG)

            zTf = bigpool.tile([128, N], F32R, tag="zTf")
            agvz = ag_out["z"].rearrange("(c p) n -> c p n", p=H2)
            for c in range(NC):
                for hh in range(2):
                    st = stg.tile([H2, 256], F32, tag="zstg")
                    nc.sync.dma_start(out=st[:], in_=agvz[c, :, hh * 256:(hh + 1) * 256])
                    nc.vector.tensor_copy(
                        zTf[:H2, c * S + hh * 256:c * S + (hh + 1) * 256], st[:])

            # ---------------- zzT + BCE partials ----------------
            # BCE input is x' = sigmoid(logits) (the reference feeds
            # recover_adj into BCEWithLogits). With x' in (0,1]:
            # ls(x') = -g, ls(-x') = -x' - g where g = ln(1 + e^(-x')).
            # bce_sum = Sg + (pw-1)*Slg + Sx - Slx.
            sx_sb = accp.tile([128, 32], F32, tag="sxacc")
            sg_sb = accp.tile([128, 32], F32, tag="sgacc")
            slx_sb = accp.tile([128, 32], F32, tag="slx")
            slg_sb = accp.tile([128, 32], F32, tag="slg")
            ones_bf = cpool.tile([128, 1], BF16, tag="ones_bf")
            nc.vector.memset(ones_bf[:], 1.0)

            labv = d_lab.rearrange("(rt p) n -> rt p n", p=128)
            for cc in range(NCC):
                lps = pl.tile([1, 512], F32, tag="pl")
                for rt in range(NRT):
                    lt = labp.tile([128, 512], BF16, tag="labt")
                    nc.sync.dma_start(out=lt[:],
                                      in_=labv[rt, :, cc * 512:(cc + 1) * 512])
                    px = pg.tile([128, 512], F32, tag="pg")
                    nc.tensor.matmul(px[:], zsT_r[:H2, rt * 128:(rt + 1) * 128],
                                     zTf[:H2, cc * 512:(cc + 1) * 512],
                                     start=True, stop=True)
                    xp = wk2.tile([128, 512], F32, tag="xp")
                    gt = wk2.tile([128, 512], F32, tag="gt")
                    scr = wk2.tile([128, 512], F32, tag="scr")
                    idx = rt * NCC + cc
                    nc.scalar.activation(xp[:], px[:], AF.Sigmoid,
                                         accum_out=sx_sb[:, idx:idx + 1])
                    nc.scalar.activation(scr[:], xp[:], AF.Exp, scale=-1.0)
                    nc.scalar.activation(gt[:], scr[:], AF.Ln, bias=1.0,
                                         accum_out=sg_sb[:, idx:idx + 1])
                    nc.vector.scalar_tensor_tensor(
                        out=scr[:], in0=lt[:], scalar=1.0, in1=xp[:],
                        op0=ALU.mult, op1=ALU.mult,
                        accum_out=slx_sb[:, idx:idx + 1])
                    nc.vector.scalar_tensor_tensor(
                        out=scr[:], in0=lt[:], scalar=1.0, in1=gt[:],
                        op0=ALU.mult, op1=ALU.mult,
                        accum_out=slg_sb[:, idx:idx + 1])
                    nc.tensor.matmul(lps[:], ones_bf[:], lt[:],
                                     start=(rt == 0), stop=(rt == NRT - 1))
                lc = wk1.tile([1, 512], F32, tag="labc1")
                nc.scalar.copy(lc[:], lps[:])
                nc.sync.dma_start(out=o_labc[:, cc * 512:(cc + 1) * 512], in_=lc[:])

            nc.sync.dma_start(out=o_sx[:], in_=sx_sb[:])
            nc.sync.dma_start(out=o_sg[:], in_=sg_sb[:])
            nc.sync.dma_start(out=o_slx[:], in_=slx_sb[:])
            nc.sync.dma_start(out=o_slg[:], in_=slg_sb[:])

            # ---------------- prior encoder + KLD ----------------
            encoder("r", d_adjp)

            kld_sb = accp.tile([128, NRT], F32, tag="kldacc")
            for rt in range(NRT):
                mu_p = muv["p"][:, rt * 2 * H2:rt * 2 * H2 + H2]
                lv_p = muv["p"][:, rt * 2 * H2 + H2:(rt + 1) * 2 * H2]
                mu_r = muv["r"][:, rt * 2 * H2:rt * 2 * H2 + H2]
                lv_r = muv["r"][:, rt * 2 * H2 + H2:(rt + 1) * 2 * H2]
                d = wk1.tile([128, H2], F32, tag="kd")
                e1 = wk1.tile([128, H2], F32, tag="ke1")
                e2 = wk1.tile([128, H2], F32, tag="ke2")
                a = wk1.tile([128, H2], F32, tag="ka")
                scr3 = wk1.tile([128, H2], F32, tag="kscr")
                nc.vector.tensor_tensor(out=d[:], in0=mu_r, in1=mu_p, op=ALU.subtract)
                nc.vector.tensor_tensor(out=d[:], in0=d[:], in1=d[:], op=ALU.mult)
                nc.scalar.activation(e1[:], lv_r, AF.Exp, scale=-1.0)
                nc.vector.tensor_tensor(out=a[:], in0=lv_p, in1=lv_r, op=ALU.subtract)
                nc.scalar.activation(e2[:], a[:], AF.Exp)
                nc.vector.tensor_tensor(out=d[:], in0=d[:], in1=e1[:], op=ALU.mult)
                nc.vector.tensor_tensor(out=d[:], in0=d[:], in1=e2[:], op=ALU.add)
                nc.vector.tensor_tensor(out=d[:], in0=d[:], in1=a[:], op=ALU.subtract)
                nc.vector.tensor_scalar(out=scr3[:], in0=d[:], scalar1=1.0,
                                        scalar2=0.0, op0=ALU.mult, op1=ALU.add,
                                        accum_out=kld_sb[:, rt:rt + 1])
            nc.sync.dma_start(out=o_kld[:], in_=kld_sb[:])

    nc.compile()
    return nc


def _get_compiled():
    global _COMPILED
    if _COMPILED is None:
        _COMPILED = _build()
    return _COMPILED


# fp32 sigmoid(logit) == 1.0 iff logit > T_SAT (the fp32 saturation boundary
# sits in (16.63553, 16.635555); any threshold inside classifies identically).
T_SAT = 16.635544


def kernel(ns_emb, adj, adj_prior, condition, labels, eps,
           post_Wh, post_Wq, post_Wk, post_Wv, post_Wo, post_Wmu, post_Wvar,
           prior_Wh, prior_Wq, prior_Wk, prior_Wv, prior_Wo, prior_Wmu, prior_Wvar,
           Wmap):
    from concourse.bass_utils import run_bass_kernel_spmd

    nc = _get_compiled()
    n = N
    f32 = np.float32
    lab_bf = np.asarray(labels, f32).astype(ml_dtypes.bfloat16)

    shared = {
        "cond": np.ascontiguousarray(np.asarray(condition, f32)[0]),
        "Wh_p": np.asarray(post_Wh, f32), "Wq_p": np.asarray(post_Wq, f32),
        "Wk_p": np.asarray(post_Wk, f32), "Wv_p": np.asarray(post_Wv, f32),
        "Wo_p": np.asarray(post_Wo, f32),
        "Wmv_p": np.ascontiguousarray(np.concatenate(
            [np.asarray(post_Wmu, f32), np.asarray(post_Wvar, f32)], axis=1)),
        "Wh_r": np.asarray(prior_Wh, f32), "Wq_r": np.asarray(prior_Wq, f32),
        "Wk_r": np.asarray(prior_Wk, f32), "Wv_r": np.asarray(prior_Wv, f32),
        "Wo_r": np.asarray(prior_Wo, f32),
        "Wmv_r": np.ascontiguousarray(np.concatenate(
            [np.asarray(prior_Wmu, f32), np.asarray(prior_Wvar, f32)], axis=1)),
        "Wmap": np.asarray(Wmap, f32),
    }
    in_maps = []
    for c in range(NC):
        r0, r1 = c * S, (c + 1) * S
        m = dict(shared)
        m["adj_s"] = np.ascontiguousarray(np.asarray(adj, f32)[r0:r1])
        m["adjp_s"] = np.ascontiguousarray(np.asarray(adj_prior, f32)[r0:r1])
        m["lab_s"] = np.ascontiguousarray(lab_bf[r0:r1])
        m["ns_s"] = np.ascontiguousarray(np.asarray(ns_emb, f32)[r0:r1])
        m["eps_s"] = np.ascontiguousarray(np.asarray(eps, f32)[r0:r1])
        in_maps.append(m)

    global _last_in_maps
    _last_in_maps = in_maps
    res = run_bass_kernel_spmd(nc, in_maps, list(range(NC)))
    R = res.results

    z = np.concatenate([R[c]["z_out"] for c in range(NC)], axis=0)
    m_full = np.concatenate([R[c]["m_out"] for c in range(NC)], axis=0)

    # ---- top-512: first 512 flat triu indices whose fp32 sigmoid saturates
    z64 = z.astype(np.float64)
    sel_i, sel_j, count = [], [], 0
    for r in range(n - 1):
        lg = z64[r] @ z64.T
        cols = np.nonzero((np.arange(n) > r) & (lg > T_SAT))[0]
        take = cols[:MAX_K - count]
        sel_i.extend([r] * len(take))
        sel_j.extend(take.tolist())
        count += len(take)
        if count >= MAX_K:
            break
    if count < MAX_K:
        # general fallback: exact fp32 replication of the reference top_k
        lg32 = z.astype(f32) @ z.astype(f32).T
        sig = (1.0 / (1.0 + np.exp(-lg32, dtype=f32))).astype(f32)
        triu = np.triu(sig, k=1).ravel()
        order = np.lexsort((np.arange(triu.size), -triu))[:MAX_K]
        sel_i = (order // n).tolist()
        sel_j = (order % n).tolist()
    sel_i = np.asarray(sel_i[:MAX_K])
    sel_j = np.asarray(sel_j[:MAX_K])
    relations = (m_full[sel_i] + m_full[sel_j]).astype(f32)
    rel_num = (n * n - n) / 2.0
    rel_mask = np.arange(MAX_K) >= rel_num

    # ---- losses from partials
    def tot(name):
        return float(sum(R[c][name].astype(np.float64).sum() for c in range(NC)))

    Sx, Sg, Slx, Slg = (tot(x) for x in ("sx", "sg", "slx", "slg"))
    lab_sum = tot("labcol")
    pos_weight = (n * n - lab_sum + n) / (lab_sum - n + 0.01)
    bce_sum = Sg + (pos_weight - 1.0) * Slg + Sx - Slx
    norm = (n * n) / (n * n - lab_sum + n)
    recons_loss = np.float32(norm * bce_sum / (n * n))

    kld_tot = tot("kldp") - n * H2
    kld = np.float32(0.5 / n * kld_tot / n)

    return relations, rel_mask, recons_loss, kld
